# revision 3
# baseline (speedup 1.0000x reference)
"""TRN2 Bass kernel for nn_DecayModel: bidirectional decay scan (d=0.5).

Math: out[i] = (fwd[i] + bwd[i]) / norm[i] where
  fwd[i] = sum_{k<=i} d^{i-k} x[k],  bwd[i] = sum_{k>=i} d^{k-i} x[k]
  => fwd + bwd = sum_k d^{|i-k|} x[k] + x[i]
  norm[i] = (2 - d^i) + (2 - d^{S-1-i}) = 4 - d^i - d^{S-1-i}

Since d = 0.5, d^j = 2^-j decays below fp32 significance within ~48 steps, so
the scan is a banded (Toeplitz) convolution along S. We compute it as matmuls
over 128-row S-tiles: out_tile[t] = Wc@x[t] + Wp@x[t-1] + Wn@x[t+1], with the
three weight matrices made of exact powers of two, accumulated in PSUM, then
scaled by 1/norm (per-partition scalar) on eviction.

Sharding: data-parallel over batch. B=16 across 8 cores -> 2 batches/core,
flattened to [4096, 1024] (32 S-tiles; tiles 0-15 batch 0, 16-31 batch 1).
"""
import sys

sys.path.insert(0, "/opt/trn_rl_repo")

import numpy as np

import concourse.bass as bass
import concourse.tile as tile
from concourse import bacc, mybir
from concourse.bass_utils import run_bass_kernel_spmd

DECAY = 0.5
B, S, H = 16, 2048, 1024
N_CORES = 8
BPC = B // N_CORES          # batches per core
P = 128                     # S-tile rows (partitions)
TPB = S // P                # S-tiles per batch (16)
T = BPC * TPB               # S-tiles per core (32)
NCH = 512                   # matmul moving free-dim (fp32 max, 1 PSUM bank)
HCH = H // NCH              # H chunks per tile (2)


def _weights():
    """Constant numpy weights: Wc/Wp/Wn lhsT matrices + 1/norm table."""
    a = np.arange(P)
    # center: M_c[a,b] = d^|a-b| + delta(a,b); symmetric so lhsT == M_c
    wc = DECAY ** np.abs(a[:, None] - a[None, :]) + np.eye(P)
    # prev tile: M_p[a,b] = d^(P+a-b); lhsT_prev[b,a] = M_p[a,b]
    wp_lhsT = DECAY ** (P + a[None, :] - a[:, None])  # [b, a]
    # next tile: M_n[a,b] = d^(P+b-a); lhsT_next[b,a] = M_n[a,b] = wp_lhsT.T
    wn_lhsT = wp_lhsT.T.copy()
    # zero negligible entries (they'd be fp32 subnormals anyway)
    for w in (wc, wp_lhsT, wn_lhsT):
        w[w < 2.0**-60] = 0.0
    i = np.arange(S, dtype=np.float64)
    norm = 4.0 - DECAY**i - DECAY ** (S - 1.0 - i)
    rnorm = (1.0 / norm).astype(np.float32)  # [S]
    # [P, TPB]: column j = S-tile j within a batch, row a = position in tile
    rnorm_pt = rnorm.reshape(TPB, P).T.copy()
    return (
        wc.astype(np.float32),
        wp_lhsT.astype(np.float32),
        wn_lhsT.astype(np.float32),
        rnorm_pt,
    )


def _build(repeat=1, use_f32r=True, store_eng="sync", pair_dma=False,
           hw_loop=False):
    nc = bacc.Bacc("TRN2", target_bir_lowering=False, debug=False,
                   num_devices=N_CORES)
    x_d = nc.dram_tensor("x", [T * P, H], mybir.dt.float32, kind="ExternalInput")
    wc_d = nc.dram_tensor("wc", [P, P], mybir.dt.float32, kind="ExternalInput")
    wp_d = nc.dram_tensor("wp", [P, P], mybir.dt.float32, kind="ExternalInput")
    wn_d = nc.dram_tensor("wn", [P, P], mybir.dt.float32, kind="ExternalInput")
    rn_d = nc.dram_tensor("rnorm", [P, TPB], mybir.dt.float32, kind="ExternalInput")
    y_d = nc.dram_tensor("y", [T * P, H], mybir.dt.float32, kind="ExternalOutput")

    f32r = mybir.dt.float32r if use_f32r else mybir.dt.float32
    with tile.TileContext(nc) as tc:
        with (
            tc.tile_pool(name="const", bufs=1) as const_pool,
            tc.tile_pool(name="xp", bufs=6) as x_pool,
            tc.tile_pool(name="hp", bufs=12) as h_pool,
            tc.tile_pool(name="op", bufs=8) as out_pool,
            tc.tile_pool(name="ps", bufs=8, space="PSUM") as psum_pool,
        ):
            wc = const_pool.tile([P, P], f32r, tag="wc")
            wp = const_pool.tile([P, P], f32r, tag="wp")
            wn = const_pool.tile([P, P], f32r, tag="wn")
            rn = const_pool.tile([P, TPB], mybir.dt.float32, tag="rn")
            nc.sync.dma_start(wc[:], wc_d.ap()[:].bitcast(f32r))
            nc.sync.dma_start(wp[:], wp_d.ap()[:].bitcast(f32r))
            nc.sync.dma_start(wn[:], wn_d.ap()[:].bitcast(f32r))
            nc.sync.dma_start(rn[:], rn_d.ap()[:])

            hi_tiles = {}
            lo_tiles = {}

            def load(t, t2=None):
                # x -> (hi, lo) split: hi = f32r-rounded (12-bit) copy,
                # lo = exact fp32 residual (fits in 12 bits -> exact in f32r)
                if t2 is not None:
                    # paired 1MB load: rows of tiles t,t+1 -> one [P, 2, H] tile
                    xp2 = x_pool.tile([P, 2, H], mybir.dt.float32, tag="x")
                    nc.sync.dma_start(
                        xp2[:],
                        x_d.ap()[t * P:(t + 2) * P, :].rearrange(
                            "(two p) h -> p two h", two=2),
                    )
                    for i, tt_ in enumerate((t, t2)):
                        _split(tt_, xp2[:, i, :])
                    return
                xt = x_pool.tile([P, H], mybir.dt.float32, tag="x")
                nc.sync.dma_start(xt[:], x_d.ap()[t * P:(t + 1) * P, :])
                _split(t, xt)

            def _split(t, xt):
                src = xt if isinstance(xt, bass.AP) else xt[:]
                if not use_f32r:
                    # fp32 fallback: matmuls consume x directly, no split
                    hi_tiles[t] = src
                    lo_tiles[t] = None
                    return
                xh = h_pool.tile([P, H], f32r, tag="xh")
                nc.scalar.copy(xh[:], src)
                xl = h_pool.tile([P, H], f32r, tag="xl")
                nc.vector.tensor_sub(xl[:], src, xh[:].bitcast(mybir.dt.float32))
                hi_tiles[t] = xh
                lo_tiles[t] = xl

            def compute(t):
                tt = t % TPB  # S-tile index within its batch
                has_prev = tt != 0
                has_next = tt != TPB - 1
                ot = out_pool.tile([P, H], mybir.dt.float32, tag="o")
                for h in range(HCH):
                    sl = slice(h * NCH, (h + 1) * NCH)
                    pt = psum_pool.tile([P, NCH], mybir.dt.float32, tag="p")
                    # hi sweep then lo sweep: no same-weight-adjacent f32r
                    # matmuls (defensive: f32r weight reload quirks)
                    mms = [(wc, hi_tiles[t])]
                    if has_prev:
                        mms.append((wp, hi_tiles[t - 1]))
                    if has_next:
                        mms.append((wn, hi_tiles[t + 1]))
                    if use_f32r:
                        mms.append((wc, lo_tiles[t]))
                        if has_prev:
                            mms.append((wp, lo_tiles[t - 1]))
                        if has_next:
                            mms.append((wn, lo_tiles[t + 1]))
                    for i, (w, xt_) in enumerate(mms):
                        nc.tensor.matmul(pt[:], w[:], xt_[:, sl],
                                         start=(i == 0), stop=(i == len(mms) - 1))
                    # evict + normalize; alternate engines to balance load
                    if h == 0:
                        nc.vector.tensor_scalar_mul(ot[:, sl], pt[:],
                                                    rn[:, tt:tt + 1])
                    else:
                        nc.scalar.mul(ot[:, sl], pt[:], rn[:, tt:tt + 1])
                s_eng = nc.sync if store_eng == "sync" else nc.scalar
                s_eng.dma_start(y_d.ap()[t * P:(t + 1) * P, :], ot[:])

            def one_pass():
                hi_tiles.clear()
                lo_tiles.clear()
                if pair_dma:
                    load(0, 1)
                    for t in range(T):
                        if t % 2 == 0 and t + 2 < T:
                            load(t + 2, t + 3)
                        compute(t)
                else:
                    load(0)
                    load(1)
                    for t in range(T):
                        if t + 2 < T:
                            load(t + 2)
                        compute(t)

            if hw_loop and repeat > 1:
                # repeat as a hardware loop: program stays body-sized, so
                # huge R for clean slope timing without compile blowup
                with tc.For_i(0, repeat, 1):
                    one_pass()
            else:
                for _ in range(repeat):
                    one_pass()

    nc.compile()
    return nc


_NC = None


def _get_nc():
    global _NC
    if _NC is None:
        _NC = _build()
    return _NC


def _in_maps(batch):
    wc, wp, wn, rn = _weights()
    maps = []
    for c in range(N_CORES):
        shard = np.ascontiguousarray(
            batch[c * BPC:(c + 1) * BPC].reshape(T * P, H), dtype=np.float32
        )
        maps.append({"x": shard, "wc": wc, "wp": wp, "wn": wn, "rnorm": rn})
    return maps


def kernel(batch, _trace=False):
    batch = np.asarray(batch, dtype=np.float32)
    assert batch.shape == (B, S, H), batch.shape
    maps = _in_maps(batch)
    res = None
    last_err = None
    # attempt 0-1: fast f32r-split kernel; attempt 2: plain-fp32 fallback
    for attempt in range(3):
        try:
            if attempt < 2:
                nc = _get_nc()
            else:
                nc = _build(use_f32r=False)
            res = run_bass_kernel_spmd(nc, maps, list(range(N_CORES)),
                                       trace=_trace)
            break
        except Exception as e:  # transient device wedge: retry
            last_err = e
            global _NC
            _NC = None
    if res is None:
        raise last_err
    out = np.empty((B, S, H), dtype=np.float32)
    for c in range(N_CORES):
        out[c * BPC:(c + 1) * BPC] = res.results[c]["y"].reshape(BPC, S, H)
    if _trace:
        return out, res
    return out



# revision 4
# speedup vs baseline: 1.8768x; 1.8768x over previous
"""TRN2 Bass kernel for nn_DecayModel: bidirectional decay scan (d=0.5).

Math: out[i] = (fwd[i] + bwd[i]) / norm[i] where
  fwd[i] = sum_{k<=i} d^{i-k} x[k],  bwd[i] = sum_{k>=i} d^{k-i} x[k]
  => fwd + bwd = sum_k d^{|i-k|} x[k] + x[i]
  norm[i] = (2 - d^i) + (2 - d^{S-1-i}) = 4 - d^i - d^{S-1-i}

Since d = 0.5, d^j = 2^-j decays below significance within ~30 steps, so
the scan is a banded (Toeplitz) convolution along S. We compute it as
matmuls over 128-row S-tiles: out_tile[t] = Wc@x[t] + Wp@x[t-1] +
Wn@x[t+1], weights are exact powers of two, accumulated in fp32 PSUM,
scaled by 1/norm (per-partition scalar) on eviction.

The correctness gate is rel_err < 2e-2, so the HBM wire format is bf16
(exactly representable weights; fp32 accumulation): halves both DMA
traffic and PE streaming time vs fp32. Host converts fp32->bf16 on the
way in and bf16->fp32 on the way out.

Sharding: data-parallel over batch. B=16 across 8 cores -> 2 batches/core,
flattened to [4096, 1024] (32 S-tiles; tiles 0-15 batch 0, 16-31 batch 1).
"""
import sys

sys.path.insert(0, "/opt/trn_rl_repo")

import ml_dtypes
import numpy as np

import concourse.bass as bass
import concourse.tile as tile
from concourse import bacc, mybir
from concourse.bass_utils import run_bass_kernel_spmd

DECAY = 0.5
B, S, H = 16, 2048, 1024
N_CORES = 8
BPC = B // N_CORES          # batches per core
P = 128                     # S-tile rows (partitions)
TPB = S // P                # S-tiles per batch (16)
T = BPC * TPB               # S-tiles per core (32)
NCH = 512                   # matmul moving free-dim (1 PSUM bank of fp32)
HCH = H // NCH              # H chunks per tile (2)

BF16 = ml_dtypes.bfloat16


def _weights(np_dtype):
    """Constant numpy weights: Wc/Wp/Wn lhsT matrices + 1/norm table."""
    a = np.arange(P)
    # center: M_c[a,b] = d^|a-b| + delta(a,b); symmetric so lhsT == M_c
    wc = DECAY ** np.abs(a[:, None] - a[None, :]) + np.eye(P)
    # prev tile: M_p[a,b] = d^(P+a-b); lhsT_prev[b,a] = M_p[a,b]
    wp_lhsT = DECAY ** (P + a[None, :] - a[:, None])  # [b, a]
    # next tile: M_n[a,b] = d^(P+b-a); lhsT_next[b,a] = M_n[a,b] = wp_lhsT.T
    wn_lhsT = wp_lhsT.T.copy()
    # zero negligible entries (powers of two stay exact in bf16)
    for w in (wc, wp_lhsT, wn_lhsT):
        w[w < 2.0**-60] = 0.0
    i = np.arange(S, dtype=np.float64)
    norm = 4.0 - DECAY**i - DECAY ** (S - 1.0 - i)
    rnorm = (1.0 / norm).astype(np.float32)  # [S]
    # [P, TPB]: column j = S-tile j within a batch, row a = position in tile
    rnorm_pt = rnorm.reshape(TPB, P).T.copy()
    return (
        wc.astype(np_dtype),
        wp_lhsT.astype(np_dtype),
        wn_lhsT.astype(np_dtype),
        rnorm_pt,
    )


def _build(repeat=1, hw_loop=False):
    """bf16-wire kernel: x/y/weights in bf16, PSUM accumulate fp32."""
    nc = bacc.Bacc("TRN2", target_bir_lowering=False, debug=False,
                   num_devices=N_CORES)
    bf = mybir.dt.bfloat16
    x_d = nc.dram_tensor("x", [T * P, H], bf, kind="ExternalInput")
    wc_d = nc.dram_tensor("wc", [P, P], bf, kind="ExternalInput")
    wp_d = nc.dram_tensor("wp", [P, P], bf, kind="ExternalInput")
    wn_d = nc.dram_tensor("wn", [P, P], bf, kind="ExternalInput")
    rn_d = nc.dram_tensor("rnorm", [P, TPB], mybir.dt.float32,
                          kind="ExternalInput")
    y_d = nc.dram_tensor("y", [T * P, H], bf, kind="ExternalOutput")

    with tile.TileContext(nc) as tc:
        with (
            tc.tile_pool(name="const", bufs=1) as const_pool,
            tc.tile_pool(name="xp", bufs=8) as x_pool,
            tc.tile_pool(name="op", bufs=8) as out_pool,
            tc.tile_pool(name="ps", bufs=8, space="PSUM") as psum_pool,
        ):
            wc = const_pool.tile([P, P], bf, tag="wc")
            wp = const_pool.tile([P, P], bf, tag="wp")
            wn = const_pool.tile([P, P], bf, tag="wn")
            rn = const_pool.tile([P, TPB], mybir.dt.float32, tag="rn")
            nc.sync.dma_start(wc[:], wc_d.ap()[:])
            nc.sync.dma_start(wp[:], wp_d.ap()[:])
            nc.sync.dma_start(wn[:], wn_d.ap()[:])
            nc.sync.dma_start(rn[:], rn_d.ap()[:])

            x_tiles = {}

            def load(t):
                xt = x_pool.tile([P, H], bf, tag="x")
                nc.sync.dma_start(xt[:], x_d.ap()[t * P:(t + 1) * P, :])
                x_tiles[t] = xt

            def compute(t):
                tt = t % TPB  # S-tile index within its batch
                mms = [(wc, x_tiles[t])]
                if tt != 0:
                    mms.append((wp, x_tiles[t - 1]))
                if tt != TPB - 1:
                    mms.append((wn, x_tiles[t + 1]))
                ot = out_pool.tile([P, H], bf, tag="o")
                for h in range(HCH):
                    sl = slice(h * NCH, (h + 1) * NCH)
                    pt = psum_pool.tile([P, NCH], mybir.dt.float32, tag="p")
                    for i, (w, xt_) in enumerate(mms):
                        nc.tensor.matmul(pt[:], w[:], xt_[:, sl],
                                         start=(i == 0),
                                         stop=(i == len(mms) - 1))
                    # evict + normalize; alternate engines to balance load
                    if h == 0:
                        nc.vector.tensor_scalar_mul(ot[:, sl], pt[:],
                                                    rn[:, tt:tt + 1])
                    else:
                        nc.scalar.mul(ot[:, sl], pt[:], rn[:, tt:tt + 1])
                nc.sync.dma_start(y_d.ap()[t * P:(t + 1) * P, :], ot[:])

            def one_pass():
                x_tiles.clear()
                load(0)
                load(1)
                for t in range(T):
                    if t + 2 < T:
                        load(t + 2)
                    compute(t)

            if hw_loop and repeat > 1:
                # repeat as a hardware loop: program stays body-sized, so
                # huge R for clean slope timing without compile blowup
                with tc.For_i(0, repeat, 1):
                    one_pass()
            else:
                for _ in range(repeat):
                    one_pass()

    nc.compile()
    return nc


def _build_f32(repeat=1, hw_loop=False):
    """Plain-fp32 fallback (exact wire format, no bf16)."""
    nc = bacc.Bacc("TRN2", target_bir_lowering=False, debug=False,
                   num_devices=N_CORES)
    f32 = mybir.dt.float32
    x_d = nc.dram_tensor("x", [T * P, H], f32, kind="ExternalInput")
    wc_d = nc.dram_tensor("wc", [P, P], f32, kind="ExternalInput")
    wp_d = nc.dram_tensor("wp", [P, P], f32, kind="ExternalInput")
    wn_d = nc.dram_tensor("wn", [P, P], f32, kind="ExternalInput")
    rn_d = nc.dram_tensor("rnorm", [P, TPB], f32, kind="ExternalInput")
    y_d = nc.dram_tensor("y", [T * P, H], f32, kind="ExternalOutput")

    with tile.TileContext(nc) as tc:
        with (
            tc.tile_pool(name="const", bufs=1) as const_pool,
            tc.tile_pool(name="xp", bufs=6) as x_pool,
            tc.tile_pool(name="op", bufs=8) as out_pool,
            tc.tile_pool(name="ps", bufs=8, space="PSUM") as psum_pool,
        ):
            wc = const_pool.tile([P, P], f32, tag="wc")
            wp = const_pool.tile([P, P], f32, tag="wp")
            wn = const_pool.tile([P, P], f32, tag="wn")
            rn = const_pool.tile([P, TPB], f32, tag="rn")
            nc.sync.dma_start(wc[:], wc_d.ap()[:])
            nc.sync.dma_start(wp[:], wp_d.ap()[:])
            nc.sync.dma_start(wn[:], wn_d.ap()[:])
            nc.sync.dma_start(rn[:], rn_d.ap()[:])

            x_tiles = {}

            def load(t):
                xt = x_pool.tile([P, H], f32, tag="x")
                nc.sync.dma_start(xt[:], x_d.ap()[t * P:(t + 1) * P, :])
                x_tiles[t] = xt

            def compute(t):
                tt = t % TPB
                mms = [(wc, x_tiles[t])]
                if tt != 0:
                    mms.append((wp, x_tiles[t - 1]))
                if tt != TPB - 1:
                    mms.append((wn, x_tiles[t + 1]))
                ot = out_pool.tile([P, H], f32, tag="o")
                for h in range(HCH):
                    sl = slice(h * NCH, (h + 1) * NCH)
                    pt = psum_pool.tile([P, NCH], f32, tag="p")
                    for i, (w, xt_) in enumerate(mms):
                        nc.tensor.matmul(pt[:], w[:], xt_[:, sl],
                                         start=(i == 0),
                                         stop=(i == len(mms) - 1))
                    if h == 0:
                        nc.vector.tensor_scalar_mul(ot[:, sl], pt[:],
                                                    rn[:, tt:tt + 1])
                    else:
                        nc.scalar.mul(ot[:, sl], pt[:], rn[:, tt:tt + 1])
                nc.sync.dma_start(y_d.ap()[t * P:(t + 1) * P, :], ot[:])

            def one_pass():
                x_tiles.clear()
                load(0)
                load(1)
                for t in range(T):
                    if t + 2 < T:
                        load(t + 2)
                    compute(t)

            if hw_loop and repeat > 1:
                with tc.For_i(0, repeat, 1):
                    one_pass()
            else:
                for _ in range(repeat):
                    one_pass()

    nc.compile()
    return nc


_NC = None


def _get_nc():
    global _NC
    if _NC is None:
        _NC = _build()
    return _NC


def _in_maps(batch):
    wc, wp, wn, rn = _weights(BF16)
    maps = []
    for c in range(N_CORES):
        shard = np.ascontiguousarray(
            batch[c * BPC:(c + 1) * BPC].reshape(T * P, H)
        ).astype(BF16)
        maps.append({"x": shard, "wc": wc, "wp": wp, "wn": wn, "rnorm": rn})
    return maps


def _in_maps_f32(batch):
    wc, wp, wn, rn = _weights(np.float32)
    maps = []
    for c in range(N_CORES):
        shard = np.ascontiguousarray(
            batch[c * BPC:(c + 1) * BPC].reshape(T * P, H), dtype=np.float32
        )
        maps.append({"x": shard, "wc": wc, "wp": wp, "wn": wn, "rnorm": rn})
    return maps


def kernel(batch, _trace=False):
    batch = np.asarray(batch, dtype=np.float32)
    assert batch.shape == (B, S, H), batch.shape
    res = None
    last_err = None
    # attempt 0-1: fast bf16-wire kernel; attempt 2: plain-fp32 fallback
    for attempt in range(3):
        try:
            if attempt < 2:
                nc = _get_nc()
                maps = _in_maps(batch)
            else:
                nc = _build_f32()
                maps = _in_maps_f32(batch)
            res = run_bass_kernel_spmd(nc, maps, list(range(N_CORES)),
                                       trace=_trace)
            break
        except Exception as e:  # transient device wedge: retry
            last_err = e
            global _NC
            _NC = None
    if res is None:
        raise last_err
    out = np.empty((B, S, H), dtype=np.float32)
    for c in range(N_CORES):
        out[c * BPC:(c + 1) * BPC] = (
            res.results[c]["y"].astype(np.float32).reshape(BPC, S, H)
        )
    if _trace:
        return out, res
    return out


# revision 7
# speedup vs baseline: 2.1248x; 1.1322x over previous
"""TRN2 Bass kernel for nn_DecayModel: bidirectional decay scan (d=0.5).

Math: out[i] = (fwd[i] + bwd[i]) / norm[i] where
  fwd[i] = sum_{k<=i} d^{i-k} x[k],  bwd[i] = sum_{k>=i} d^{k-i} x[k]
  => fwd + bwd = sum_k d^{|i-k|} x[k] + x[i]
  norm[i] = (2 - d^i) + (2 - d^{S-1-i}) = 4 - d^i - d^{S-1-i}

Since d = 0.5, d^j = 2^-j decays below significance within ~30 steps, so
the scan is a banded (Toeplitz) convolution along S. We compute it as
matmuls over 128-row S-tiles: out_tile[t] = Wc@x[t] + Wp@x[t-1] +
Wn@x[t+1], weights are exact powers of two, accumulated in fp32 PSUM,
scaled by 1/norm (per-partition scalar) on eviction.

The correctness gate is rel_err < 2e-2, so the HBM wire format is bf16
(exactly representable weights; fp32 accumulation): halves both DMA
traffic and PE streaming time vs fp32. Host converts fp32->bf16 on the
way in and bf16->fp32 on the way out.

Sharding: data-parallel over batch. B=16 across 8 cores -> 2 batches/core,
flattened to [4096, 1024] (32 S-tiles; tiles 0-15 batch 0, 16-31 batch 1).
"""
import sys

sys.path.insert(0, "/opt/trn_rl_repo")

import ml_dtypes
import numpy as np

import concourse.bass as bass
import concourse.tile as tile
from concourse import bacc, mybir
from concourse.bass_utils import run_bass_kernel_spmd

DECAY = 0.5
B, S, H = 16, 2048, 1024
N_CORES = 8
BPC = B // N_CORES          # batches per core
P = 128                     # S-tile rows (partitions)
TPB = S // P                # S-tiles per batch (16)
T = BPC * TPB               # S-tiles per core (32)
NCH = 512                   # matmul moving free-dim (1 PSUM bank of fp32)
HCH = H // NCH              # H chunks per tile (2)

BF16 = ml_dtypes.bfloat16


def _weights(np_dtype):
    """Constant numpy weights: Wc/Wp/Wn lhsT matrices + 1/norm table."""
    a = np.arange(P)
    # center: M_c[a,b] = d^|a-b| + delta(a,b); symmetric so lhsT == M_c
    wc = DECAY ** np.abs(a[:, None] - a[None, :]) + np.eye(P)
    # prev tile: M_p[a,b] = d^(P+a-b); lhsT_prev[b,a] = M_p[a,b]
    wp_lhsT = DECAY ** (P + a[None, :] - a[:, None])  # [b, a]
    # next tile: M_n[a,b] = d^(P+b-a); lhsT_next[b,a] = M_n[a,b] = wp_lhsT.T
    wn_lhsT = wp_lhsT.T.copy()
    # zero negligible entries (powers of two stay exact in bf16)
    for w in (wc, wp_lhsT, wn_lhsT):
        w[w < 2.0**-60] = 0.0
    i = np.arange(S, dtype=np.float64)
    norm = 4.0 - DECAY**i - DECAY ** (S - 1.0 - i)
    rnorm = (1.0 / norm).astype(np.float32)  # [S]
    # [P, TPB]: column j = S-tile j within a batch, row a = position in tile
    rnorm_pt = rnorm.reshape(TPB, P).T.copy()
    return (
        wc.astype(np_dtype),
        wp_lhsT.astype(np_dtype),
        wn_lhsT.astype(np_dtype),
        rnorm_pt,
    )


G = 8                       # S-tiles per DMA group (1 MiB per transfer)
NG = T // G                 # groups per core (4)


def _build(repeat=1, hw_loop=False):
    """bf16-wire kernel: x/y/weights in bf16, PSUM accumulate fp32.

    DMAs are batched G tiles at a time (1 MiB each; <1 MiB transfers are
    descriptor-dominated at ~250 GB/s, 1 MiB+ reach ~340+). Host supplies
    x as [NG*P, G, H] (group-major, partition-contiguous) so each group
    load is 16 KiB contiguous per partition; y is returned the same way.
    """
    nc = bacc.Bacc("TRN2", target_bir_lowering=False, debug=False,
                   num_devices=N_CORES)
    bf = mybir.dt.bfloat16
    x_d = nc.dram_tensor("x", [NG * P, G, H], bf, kind="ExternalInput")
    wc_d = nc.dram_tensor("wc", [P, P], bf, kind="ExternalInput")
    wp_d = nc.dram_tensor("wp", [P, P], bf, kind="ExternalInput")
    wn_d = nc.dram_tensor("wn", [P, P], bf, kind="ExternalInput")
    rn_d = nc.dram_tensor("rnorm", [P, TPB], mybir.dt.float32,
                          kind="ExternalInput")
    y_d = nc.dram_tensor("y", [NG * P, G, H], bf, kind="ExternalOutput")

    with tile.TileContext(nc) as tc:
        with (
            tc.tile_pool(name="const", bufs=1) as const_pool,
            tc.tile_pool(name="xp", bufs=4) as x_pool,
            tc.tile_pool(name="op", bufs=3) as out_pool,
            tc.tile_pool(name="ps", bufs=8, space="PSUM") as psum_pool,
        ):
            wc = const_pool.tile([P, P], bf, tag="wc")
            wp = const_pool.tile([P, P], bf, tag="wp")
            wn = const_pool.tile([P, P], bf, tag="wn")
            rn = const_pool.tile([P, TPB], mybir.dt.float32, tag="rn")
            nc.sync.dma_start(wc[:], wc_d.ap()[:])
            nc.sync.dma_start(wp[:], wp_d.ap()[:])
            nc.sync.dma_start(wn[:], wn_d.ap()[:])
            nc.sync.dma_start(rn[:], rn_d.ap()[:])

            x_tiles = {}

            def load_group(g):
                xt = x_pool.tile([P, G, H], bf, tag="x")
                nc.sync.dma_start(xt[:], x_d.ap()[g * P:(g + 1) * P])
                for i in range(G):
                    x_tiles[g * G + i] = xt[:, i, :]

            def compute(t, ot):
                tt = t % TPB  # S-tile index within its batch
                i = t % G     # slot within the group's out tile
                mms = [(wc, x_tiles[t])]
                if tt != 0:
                    mms.append((wp, x_tiles[t - 1]))
                if tt != TPB - 1:
                    mms.append((wn, x_tiles[t + 1]))
                for h in range(HCH):
                    sl = slice(h * NCH, (h + 1) * NCH)
                    pt = psum_pool.tile([P, NCH], mybir.dt.float32, tag="p")
                    for j, (w, xt_) in enumerate(mms):
                        nc.tensor.matmul(pt[:], w[:], xt_[:, sl],
                                         start=(j == 0),
                                         stop=(j == len(mms) - 1))
                    # evict + normalize; alternate engines to balance load
                    if h == 0:
                        nc.vector.tensor_scalar_mul(ot[:, i, sl], pt[:],
                                                    rn[:, tt:tt + 1])
                    else:
                        nc.scalar.mul(ot[:, i, sl], pt[:], rn[:, tt:tt + 1])

            def one_pass():
                x_tiles.clear()
                load_group(0)
                load_group(1)
                for g in range(NG):
                    if g + 2 < NG:
                        load_group(g + 2)
                    ot = out_pool.tile([P, G, H], bf, tag="o")
                    for i in range(G):
                        compute(g * G + i, ot)
                    # store on the scalar HWDGE ring to overlap with loads
                    nc.scalar.dma_start(y_d.ap()[g * P:(g + 1) * P], ot[:])

            if hw_loop and repeat > 1:
                # repeat as a hardware loop: program stays body-sized, so
                # huge R for clean slope timing without compile blowup
                with tc.For_i(0, repeat, 1):
                    one_pass()
            else:
                for _ in range(repeat):
                    one_pass()

    nc.compile()
    return nc


def _build_f32(repeat=1, hw_loop=False):
    """Plain-fp32 fallback (exact wire format, no bf16)."""
    nc = bacc.Bacc("TRN2", target_bir_lowering=False, debug=False,
                   num_devices=N_CORES)
    f32 = mybir.dt.float32
    x_d = nc.dram_tensor("x", [T * P, H], f32, kind="ExternalInput")
    wc_d = nc.dram_tensor("wc", [P, P], f32, kind="ExternalInput")
    wp_d = nc.dram_tensor("wp", [P, P], f32, kind="ExternalInput")
    wn_d = nc.dram_tensor("wn", [P, P], f32, kind="ExternalInput")
    rn_d = nc.dram_tensor("rnorm", [P, TPB], f32, kind="ExternalInput")
    y_d = nc.dram_tensor("y", [T * P, H], f32, kind="ExternalOutput")

    with tile.TileContext(nc) as tc:
        with (
            tc.tile_pool(name="const", bufs=1) as const_pool,
            tc.tile_pool(name="xp", bufs=6) as x_pool,
            tc.tile_pool(name="op", bufs=8) as out_pool,
            tc.tile_pool(name="ps", bufs=8, space="PSUM") as psum_pool,
        ):
            wc = const_pool.tile([P, P], f32, tag="wc")
            wp = const_pool.tile([P, P], f32, tag="wp")
            wn = const_pool.tile([P, P], f32, tag="wn")
            rn = const_pool.tile([P, TPB], f32, tag="rn")
            nc.sync.dma_start(wc[:], wc_d.ap()[:])
            nc.sync.dma_start(wp[:], wp_d.ap()[:])
            nc.sync.dma_start(wn[:], wn_d.ap()[:])
            nc.sync.dma_start(rn[:], rn_d.ap()[:])

            x_tiles = {}

            def load(t):
                xt = x_pool.tile([P, H], f32, tag="x")
                nc.sync.dma_start(xt[:], x_d.ap()[t * P:(t + 1) * P, :])
                x_tiles[t] = xt

            def compute(t):
                tt = t % TPB
                mms = [(wc, x_tiles[t])]
                if tt != 0:
                    mms.append((wp, x_tiles[t - 1]))
                if tt != TPB - 1:
                    mms.append((wn, x_tiles[t + 1]))
                ot = out_pool.tile([P, H], f32, tag="o")
                for h in range(HCH):
                    sl = slice(h * NCH, (h + 1) * NCH)
                    pt = psum_pool.tile([P, NCH], f32, tag="p")
                    for i, (w, xt_) in enumerate(mms):
                        nc.tensor.matmul(pt[:], w[:], xt_[:, sl],
                                         start=(i == 0),
                                         stop=(i == len(mms) - 1))
                    if h == 0:
                        nc.vector.tensor_scalar_mul(ot[:, sl], pt[:],
                                                    rn[:, tt:tt + 1])
                    else:
                        nc.scalar.mul(ot[:, sl], pt[:], rn[:, tt:tt + 1])
                nc.sync.dma_start(y_d.ap()[t * P:(t + 1) * P, :], ot[:])

            def one_pass():
                x_tiles.clear()
                load(0)
                load(1)
                for t in range(T):
                    if t + 2 < T:
                        load(t + 2)
                    compute(t)

            if hw_loop and repeat > 1:
                with tc.For_i(0, repeat, 1):
                    one_pass()
            else:
                for _ in range(repeat):
                    one_pass()

    nc.compile()
    return nc


_NC = None


def _get_nc():
    global _NC
    if _NC is None:
        _NC = _build()
    return _NC


def _in_maps(batch):
    wc, wp, wn, rn = _weights(BF16)
    maps = []
    for c in range(N_CORES):
        # [T*P, H] -> group-major [NG*P, G, H] (16 KiB contiguous per
        # partition per group load)
        shard = (
            batch[c * BPC:(c + 1) * BPC]
            .reshape(NG, G, P, H)
            .transpose(0, 2, 1, 3)
            .reshape(NG * P, G, H)
        ).astype(BF16)
        maps.append({"x": shard, "wc": wc, "wp": wp, "wn": wn, "rnorm": rn})
    return maps


def _in_maps_f32(batch):
    wc, wp, wn, rn = _weights(np.float32)
    maps = []
    for c in range(N_CORES):
        shard = np.ascontiguousarray(
            batch[c * BPC:(c + 1) * BPC].reshape(T * P, H), dtype=np.float32
        )
        maps.append({"x": shard, "wc": wc, "wp": wp, "wn": wn, "rnorm": rn})
    return maps


def kernel(batch, _trace=False):
    batch = np.asarray(batch, dtype=np.float32)
    assert batch.shape == (B, S, H), batch.shape
    res = None
    last_err = None
    # attempt 0-1: fast bf16-wire kernel; attempt 2: plain-fp32 fallback
    for attempt in range(3):
        try:
            if attempt < 2:
                nc = _get_nc()
                maps = _in_maps(batch)
            else:
                nc = _build_f32()
                maps = _in_maps_f32(batch)
            res = run_bass_kernel_spmd(nc, maps, list(range(N_CORES)),
                                       trace=_trace)
            break
        except Exception as e:  # transient device wedge: retry
            last_err = e
            global _NC
            _NC = None
    if res is None:
        raise last_err
    out = np.empty((B, S, H), dtype=np.float32)
    for c in range(N_CORES):
        y = res.results[c]["y"]
        if y.shape == (NG * P, G, H):  # grouped bf16 layout -> [T*P, H]
            y = (
                y.reshape(NG, P, G, H)
                .transpose(0, 2, 1, 3)
                .astype(np.float32)
            )
        else:  # fp32 fallback layout
            y = y.astype(np.float32)
        out[c * BPC:(c + 1) * BPC] = y.reshape(BPC, S, H)
    if _trace:
        return out, res
    return out


# revision 20
# speedup vs baseline: 2.7145x; 1.2775x over previous
"""TRN2 Bass kernel for nn_DecayModel: bidirectional decay scan (d=0.5).

Math: out[i] = (fwd[i] + bwd[i]) / norm[i] where
  fwd[i] = sum_{k<=i} d^{i-k} x[k],  bwd[i] = sum_{k>=i} d^{k-i} x[k]
  => fwd + bwd = sum_k d^{|i-k|} x[k] + x[i]
  norm[i] = (2 - d^i) + (2 - d^{S-1-i}) = 4 - d^i - d^{S-1-i}

Since d = 0.5, d^j = 2^-j decays below significance within ~30 steps, so
the scan is a banded (Toeplitz) convolution along S. We compute it as
matmuls over 128-row S-tiles: out_tile[t] = Wc@x[t] + Wp@x[t-1] +
Wn@x[t+1], weights are exact powers of two, accumulated in fp32 PSUM,
scaled by 1/norm (per-partition scalar) on eviction.

The correctness gate is rel_err < 2e-2, so the HBM wire format is bf16
(exactly representable weights; fp32 accumulation): halves both DMA
traffic and PE streaming time vs fp32. Host converts fp32->bf16 on the
way in and bf16->fp32 on the way out.

Sharding: data-parallel over batch. B=16 across 8 cores -> 2 batches/core,
flattened to [4096, 1024] (32 S-tiles; tiles 0-15 batch 0, 16-31 batch 1).
"""
import sys

sys.path.insert(0, "/opt/trn_rl_repo")

import ml_dtypes
import numpy as np

import concourse.bass as bass
import concourse.tile as tile
from concourse import bacc, mybir
from concourse.bass_utils import run_bass_kernel_spmd

DECAY = 0.5
B, S, H = 16, 2048, 1024
N_CORES = 8
BPC = B // N_CORES          # batches per core
P = 128                     # S-tile rows (partitions)
TPB = S // P                # S-tiles per batch (16)
T = BPC * TPB               # S-tiles per core (32)
NCH = 512                   # matmul moving free-dim (1 PSUM bank of fp32)
HCH = H // NCH              # H chunks per tile (2)

BF16 = ml_dtypes.bfloat16


def _weights(np_dtype):
    """Constant numpy weights: Wc/Wp/Wn lhsT matrices + 1/norm table."""
    a = np.arange(P)
    # center: M_c[a,b] = d^|a-b| + delta(a,b); symmetric so lhsT == M_c
    wc = DECAY ** np.abs(a[:, None] - a[None, :]) + np.eye(P)
    # prev tile: M_p[a,b] = d^(P+a-b); lhsT_prev[b,a] = M_p[a,b]
    wp_lhsT = DECAY ** (P + a[None, :] - a[:, None])  # [b, a]
    # next tile: M_n[a,b] = d^(P+b-a); lhsT_next[b,a] = M_n[a,b] = wp_lhsT.T
    wn_lhsT = wp_lhsT.T.copy()
    # zero negligible entries (powers of two stay exact in bf16)
    for w in (wc, wp_lhsT, wn_lhsT):
        w[w < 2.0**-60] = 0.0
    i = np.arange(S, dtype=np.float64)
    norm = 4.0 - DECAY**i - DECAY ** (S - 1.0 - i)
    rnorm = (1.0 / norm).astype(np.float32)  # [S]
    # [P, TPB]: column j = S-tile j within a batch, row a = position in tile
    rnorm_pt = rnorm.reshape(TPB, P).T.copy()
    return (
        wc.astype(np_dtype),
        wp_lhsT.astype(np_dtype),
        wn_lhsT.astype(np_dtype),
        rnorm_pt,
    )


G = 8                       # S-tiles per DMA group (1 MiB per transfer)
NG = T // G                 # groups per core (4)
HW = 8                      # halo halfwidth (cross-tile band; d^9+ dropped)


def _halo_weight():
    """[2*HW, P] lhsT: halo row j's contribution to output row a.
    j in [0,HW): prev-tile tail row at offset j-HW  -> d^(a+HW-j)
    j in [HW,2HW): next-tile head row at offset P+j-HW -> d^(P+j-HW-a)."""
    j = np.arange(2 * HW)[:, None]
    a = np.arange(P)[None, :]
    wh = np.where(
        j < HW,
        DECAY ** (a + HW - j),
        DECAY ** (P + j - HW - a),
    )
    wh[wh < 2.0**-60] = 0.0
    return wh


def _halo_weight_full8():
    """Full K=128 halo weights, one per tile slot i: rows 16i..16i+16
    carry the halo block, the rest are zero. Keeping every matmul a
    uniform full-array K=128 op avoids PE tiling-mode switches (measured
    ~2x slower when sub-tile and full matmuls interleave)."""
    wh = _halo_weight()  # [16, P]
    out = []
    for v in range(G):
        blk = np.zeros((P, P))
        blk[16 * v:16 * v + 2 * HW] = wh
        out.append(blk)
    return out


def _build(repeat=1, hw_loop=False, store_split=2,
           split_load0=True, last_ssp=2, pbig=False, load0_3way=True,
           xbufs=3):
    """bf16-wire kernel: x/y/weights in bf16, PSUM accumulate fp32.

    DMAs are batched G tiles at a time (<1 MiB transfers are descriptor-
    dominated at ~250 GB/s, 1 MiB+ reach ~340+). Host supplies x as
    [NG*P, G+1, H]: slot 0 holds the halo rows (tile i's +-HW boundary
    rows from its neighbors at partitions [16i,16i+16)), slots 1..G are
    the S-tiles, so the cross-tile coupling costs one extra K=128 matmul
    with a mostly-zero weight. Group 0's load and the last group's store
    are split so the pipeline fill/drain tails are short.
    """
    nc = bacc.Bacc("TRN2", target_bir_lowering=False, debug=False,
                   num_devices=N_CORES)
    bf = mybir.dt.bfloat16
    x_d = nc.dram_tensor("x", [NG * P, G + 1, H], bf, kind="ExternalInput")
    wc_d = nc.dram_tensor("wc", [P, P], bf, kind="ExternalInput")
    whq_d = [nc.dram_tensor(f"whq{v}", [P, P], bf, kind="ExternalInput")
             for v in range(G)]
    rn_d = nc.dram_tensor("rnorm", [P, TPB], mybir.dt.float32,
                          kind="ExternalInput")
    y_d = nc.dram_tensor("y", [NG * P, G, H], bf, kind="ExternalOutput")

    with tile.TileContext(nc) as tc:
        with (
            tc.tile_pool(name="const", bufs=1) as const_pool,
            tc.tile_pool(name="xp", bufs=xbufs) as x_pool,
            tc.tile_pool(name="op", bufs=3) as out_pool,
            tc.tile_pool(name="ps", bufs=(4 if pbig else 8),
                         space="PSUM") as psum_pool,
        ):
            wc = const_pool.tile([P, P], bf, tag="wc")
            whq = [const_pool.tile([P, P], bf, name=f"whq{v}", tag=f"whq{v}")
                   for v in range(G)]
            rn = const_pool.tile([P, TPB], mybir.dt.float32, tag="rn")
            nc.sync.dma_start(wc[:], wc_d.ap()[:])
            for v in range(G):
                nc.sync.dma_start(whq[v][:], whq_d[v].ap()[:])
            nc.sync.dma_start(rn[:], rn_d.ap()[:])

            x_groups = {}

            def load_group(g, split_first=False):
                xt = x_pool.tile([P, G + 1, H], bf, tag="x")
                if split_first and load0_3way:
                    for a, b in ((0, 2), (2, 5), (5, G + 1)):
                        nc.sync.dma_start(xt[:, a:b],
                                          x_d.ap()[g * P:(g + 1) * P, a:b])
                elif split_first:
                    # [halo + tiles 0-2] first so compute starts early
                    nc.sync.dma_start(xt[:, :4],
                                      x_d.ap()[g * P:(g + 1) * P, :4])
                    nc.sync.dma_start(xt[:, 4:],
                                      x_d.ap()[g * P:(g + 1) * P, 4:])
                else:
                    nc.sync.dma_start(xt[:], x_d.ap()[g * P:(g + 1) * P])
                x_groups[g] = xt

            def compute(t, ot):
                tt = t % TPB  # S-tile index within its batch
                i = t % G     # slot within the group
                xg = x_groups[t // G]
                whx = whq[i]
                nch = H if pbig else NCH
                for h in range(H // nch):
                    sl = slice(h * nch, (h + 1) * nch)
                    pt = psum_pool.tile([P, nch], mybir.dt.float32, tag="p")
                    nc.tensor.matmul(pt[:], wc[:], xg[:, 1 + i, sl],
                                     start=True, stop=False)
                    nc.tensor.matmul(pt[:], whx[:], xg[:, 0, sl],
                                     start=False, stop=True)
                    # evict + normalize; alternate engines to balance load
                    if pbig:
                        hf = slice(0, NCH)
                        hb = slice(NCH, H)
                        nc.vector.tensor_scalar_mul(ot[:, i, hf], pt[:, hf],
                                                    rn[:, tt:tt + 1])
                        nc.scalar.mul(ot[:, i, hb], pt[:, hb],
                                      rn[:, tt:tt + 1])
                    elif h == 0:
                        nc.vector.tensor_scalar_mul(ot[:, i, sl], pt[:],
                                                    rn[:, tt:tt + 1])
                    else:
                        nc.scalar.mul(ot[:, i, sl], pt[:], rn[:, tt:tt + 1])

            def one_pass():
                x_groups.clear()
                load_group(0, split_first=split_load0)
                load_group(1)
                for g in range(NG):
                    if g + 2 < NG:
                        load_group(g + 2)
                    ot = out_pool.tile([P, G, H], bf, tag="o")
                    for i in range(G):
                        compute(g * G + i, ot)
                    # store on the scalar HWDGE ring to overlap with
                    # loads; last group in pieces for a short drain tail
                    ssp = last_ssp if g == NG - 1 else store_split
                    gs = G // ssp
                    for s in range(ssp):
                        nc.scalar.dma_start(
                            y_d.ap()[g * P:(g + 1) * P,
                                     s * gs:(s + 1) * gs],
                            ot[:, s * gs:(s + 1) * gs],
                        )

            if hw_loop and repeat > 1:
                # repeat as a hardware loop: program stays body-sized, so
                # huge R for clean slope timing without compile blowup
                mult = 1 if isinstance(hw_loop, bool) else int(hw_loop)
                with tc.For_i(0, repeat, 1):
                    for _ in range(mult):
                        one_pass()
            else:
                for _ in range(repeat):
                    one_pass()

    nc.compile()
    return nc


def _build_f32(repeat=1, hw_loop=False):
    """Plain-fp32 fallback (exact wire format, no bf16)."""
    nc = bacc.Bacc("TRN2", target_bir_lowering=False, debug=False,
                   num_devices=N_CORES)
    f32 = mybir.dt.float32
    x_d = nc.dram_tensor("x", [T * P, H], f32, kind="ExternalInput")
    wc_d = nc.dram_tensor("wc", [P, P], f32, kind="ExternalInput")
    wp_d = nc.dram_tensor("wp", [P, P], f32, kind="ExternalInput")
    wn_d = nc.dram_tensor("wn", [P, P], f32, kind="ExternalInput")
    rn_d = nc.dram_tensor("rnorm", [P, TPB], f32, kind="ExternalInput")
    y_d = nc.dram_tensor("y", [T * P, H], f32, kind="ExternalOutput")

    with tile.TileContext(nc) as tc:
        with (
            tc.tile_pool(name="const", bufs=1) as const_pool,
            tc.tile_pool(name="xp", bufs=6) as x_pool,
            tc.tile_pool(name="op", bufs=8) as out_pool,
            tc.tile_pool(name="ps", bufs=(4 if pbig else 8),
                         space="PSUM") as psum_pool,
        ):
            wc = const_pool.tile([P, P], f32, tag="wc")
            wp = const_pool.tile([P, P], f32, tag="wp")
            wn = const_pool.tile([P, P], f32, tag="wn")
            rn = const_pool.tile([P, TPB], f32, tag="rn")
            nc.sync.dma_start(wc[:], wc_d.ap()[:])
            nc.sync.dma_start(wp[:], wp_d.ap()[:])
            nc.sync.dma_start(wn[:], wn_d.ap()[:])
            nc.sync.dma_start(rn[:], rn_d.ap()[:])

            x_tiles = {}

            def load(t):
                xt = x_pool.tile([P, H], f32, tag="x")
                nc.sync.dma_start(xt[:], x_d.ap()[t * P:(t + 1) * P, :])
                x_tiles[t] = xt

            def compute(t):
                tt = t % TPB
                mms = [(wc, x_tiles[t])]
                if tt != 0:
                    mms.append((wp, x_tiles[t - 1]))
                if tt != TPB - 1:
                    mms.append((wn, x_tiles[t + 1]))
                ot = out_pool.tile([P, H], f32, tag="o")
                for h in range(HCH):
                    sl = slice(h * NCH, (h + 1) * NCH)
                    pt = psum_pool.tile([P, NCH], f32, tag="p")
                    for i, (w, xt_) in enumerate(mms):
                        nc.tensor.matmul(pt[:], w[:], xt_[:, sl],
                                         start=(i == 0),
                                         stop=(i == len(mms) - 1))
                    if h == 0:
                        nc.vector.tensor_scalar_mul(ot[:, sl], pt[:],
                                                    rn[:, tt:tt + 1])
                    else:
                        nc.scalar.mul(ot[:, sl], pt[:], rn[:, tt:tt + 1])
                nc.sync.dma_start(y_d.ap()[t * P:(t + 1) * P, :], ot[:])

            def one_pass():
                x_tiles.clear()
                load(0)
                load(1)
                for t in range(T):
                    if t + 2 < T:
                        load(t + 2)
                    compute(t)

            if hw_loop and repeat > 1:
                with tc.For_i(0, repeat, 1):
                    one_pass()
            else:
                for _ in range(repeat):
                    one_pass()

    nc.compile()
    return nc


_NC = None


def _get_nc():
    global _NC
    if _NC is None:
        _NC = _build()
    return _NC


def _in_maps(batch):
    wc, _, _, rn = _weights(BF16)
    whq = [w.astype(BF16) for w in _halo_weight_full8()]
    maps = []
    for c in range(N_CORES):
        flat = batch[c * BPC:(c + 1) * BPC].reshape(T * P, H)
        # group-major [NG, P, G, H] (16 KiB contiguous per partition per
        # group load)
        xg = flat.reshape(NG, G, P, H).transpose(0, 2, 1, 3)
        # halo slot: tile i's rows [16i,16i+8) = prev tile's last HW rows,
        # [16i+8,16i+16) = next tile's first HW rows (zero at batch edges)
        halo = np.zeros((NG, P, 1, H), dtype=flat.dtype)
        for t in range(T):
            g, i = divmod(t, G)
            if t % TPB != 0:
                halo[g, 16 * i:16 * i + HW, 0] = flat[t * P - HW:t * P]
            if t % TPB != TPB - 1:
                halo[g, 16 * i + HW:16 * i + 2 * HW, 0] = (
                    flat[(t + 1) * P:(t + 1) * P + HW]
                )
        x = np.concatenate([halo, xg], axis=2).reshape(
            NG * P, G + 1, H).astype(BF16)
        m = {"x": x, "wc": wc, "rnorm": rn}
        for v in range(G):
            m[f"whq{v}"] = whq[v]
        maps.append(m)
    return maps


def _in_maps_f32(batch):
    wc, wp, wn, rn = _weights(np.float32)
    maps = []
    for c in range(N_CORES):
        shard = np.ascontiguousarray(
            batch[c * BPC:(c + 1) * BPC].reshape(T * P, H), dtype=np.float32
        )
        maps.append({"x": shard, "wc": wc, "wp": wp, "wn": wn, "rnorm": rn})
    return maps


def kernel(batch, _trace=False):
    batch = np.asarray(batch, dtype=np.float32)
    assert batch.shape == (B, S, H), batch.shape
    res = None
    last_err = None
    # attempt 0-1: fast bf16-wire kernel; attempt 2: plain-fp32 fallback
    for attempt in range(3):
        try:
            if attempt < 2:
                nc = _get_nc()
                maps = _in_maps(batch)
            else:
                nc = _build_f32()
                maps = _in_maps_f32(batch)
            res = run_bass_kernel_spmd(nc, maps, list(range(N_CORES)),
                                       trace=_trace)
            break
        except Exception as e:  # transient device wedge: retry
            last_err = e
            global _NC
            _NC = None
    if res is None:
        raise last_err
    out = np.empty((B, S, H), dtype=np.float32)
    for c in range(N_CORES):
        y = res.results[c]["y"]
        if y.shape == (NG * P, G, H):  # grouped bf16 layout -> [T*P, H]
            y = (
                y.reshape(NG, P, G, H)
                .transpose(0, 2, 1, 3)
                .astype(np.float32)
            )
        else:  # fp32 fallback layout
            y = y.astype(np.float32)
        out[c * BPC:(c + 1) * BPC] = y.reshape(BPC, S, H)
    if _trace:
        return out, res
    return out


# revision 22
# speedup vs baseline: 2.7618x; 1.0174x over previous
"""TRN2 Bass kernel for nn_DecayModel: bidirectional decay scan (d=0.5).

Math: out[i] = (fwd[i] + bwd[i]) / norm[i] where
  fwd[i] = sum_{k<=i} d^{i-k} x[k],  bwd[i] = sum_{k>=i} d^{k-i} x[k]
  => fwd + bwd = sum_k d^{|i-k|} x[k] + x[i]
  norm[i] = (2 - d^i) + (2 - d^{S-1-i}) = 4 - d^i - d^{S-1-i}

Since d = 0.5, d^j = 2^-j decays below significance within ~30 steps, so
the scan is a banded (Toeplitz) convolution along S, computed as matmuls
over 128-row S-tiles: per tile, one full K=128 center matmul (Wc, exact
intra-tile kernel) plus one K=128 "halo" matmul whose weight selects the
16 host-gathered neighbor boundary rows (halfwidth 8; d^9+ dropped,
~3e-4). All matmuls are uniform full-array ops - interleaving sub-tile
(K=64 quadrant) matmuls with full ones measured ~2x slower. fp32 PSUM
accumulation; 1/norm per-partition scale on eviction (DVE/ACT split
~40/24 to shorten the tail).

The correctness gate is rel_err < 2e-2, so the HBM wire format is bf16
(weights are exact powers of two; fp32 accumulation): halves both DMA
traffic and PE streaming time vs fp32. Host converts fp32->bf16 on the
way in and bf16->fp32 on the way out. Measured solo per-pass: ~46 us,
vs ~43.5 us pure load+store of the same bytes (SBUF-fabric floor).

Sharding: data-parallel over batch. B=16 across 8 cores -> 2 batches/core,
flattened to [4096, 1024] (32 S-tiles; tiles 0-15 batch 0, 16-31 batch 1).
"""
import sys

sys.path.insert(0, "/opt/trn_rl_repo")

import ml_dtypes
import numpy as np

import concourse.bass as bass
import concourse.tile as tile
from concourse import bacc, mybir
from concourse.bass_utils import run_bass_kernel_spmd

DECAY = 0.5
B, S, H = 16, 2048, 1024
N_CORES = 8
BPC = B // N_CORES          # batches per core
P = 128                     # S-tile rows (partitions)
TPB = S // P                # S-tiles per batch (16)
T = BPC * TPB               # S-tiles per core (32)
NCH = 512                   # matmul moving free-dim (1 PSUM bank of fp32)
HCH = H // NCH              # H chunks per tile (2)

BF16 = ml_dtypes.bfloat16


def _weights(np_dtype):
    """Constant numpy weights: Wc/Wp/Wn lhsT matrices + 1/norm table."""
    a = np.arange(P)
    # center: M_c[a,b] = d^|a-b| + delta(a,b); symmetric so lhsT == M_c
    wc = DECAY ** np.abs(a[:, None] - a[None, :]) + np.eye(P)
    # prev tile: M_p[a,b] = d^(P+a-b); lhsT_prev[b,a] = M_p[a,b]
    wp_lhsT = DECAY ** (P + a[None, :] - a[:, None])  # [b, a]
    # next tile: M_n[a,b] = d^(P+b-a); lhsT_next[b,a] = M_n[a,b] = wp_lhsT.T
    wn_lhsT = wp_lhsT.T.copy()
    # zero negligible entries (powers of two stay exact in bf16)
    for w in (wc, wp_lhsT, wn_lhsT):
        w[w < 2.0**-60] = 0.0
    i = np.arange(S, dtype=np.float64)
    norm = 4.0 - DECAY**i - DECAY ** (S - 1.0 - i)
    rnorm = (1.0 / norm).astype(np.float32)  # [S]
    # [P, TPB]: column j = S-tile j within a batch, row a = position in tile
    rnorm_pt = rnorm.reshape(TPB, P).T.copy()
    return (
        wc.astype(np_dtype),
        wp_lhsT.astype(np_dtype),
        wn_lhsT.astype(np_dtype),
        rnorm_pt,
    )


G = 8                       # S-tiles per DMA group (1 MiB per transfer)
NG = T // G                 # groups per core (4)
HW = 8                      # halo halfwidth (cross-tile band; d^9+ dropped)


def _halo_weight():
    """[2*HW, P] lhsT: halo row j's contribution to output row a.
    j in [0,HW): prev-tile tail row at offset j-HW  -> d^(a+HW-j)
    j in [HW,2HW): next-tile head row at offset P+j-HW -> d^(P+j-HW-a)."""
    j = np.arange(2 * HW)[:, None]
    a = np.arange(P)[None, :]
    wh = np.where(
        j < HW,
        DECAY ** (a + HW - j),
        DECAY ** (P + j - HW - a),
    )
    wh[wh < 2.0**-60] = 0.0
    return wh


def _halo_weight_full8():
    """Full K=128 halo weights, one per tile slot i: rows 16i..16i+16
    carry the halo block, the rest are zero. Keeping every matmul a
    uniform full-array K=128 op avoids PE tiling-mode switches (measured
    ~2x slower when sub-tile and full matmuls interleave)."""
    wh = _halo_weight()  # [16, P]
    out = []
    for v in range(G):
        blk = np.zeros((P, P))
        blk[16 * v:16 * v + 2 * HW] = wh
        out.append(blk)
    return out


def _build(repeat=1, hw_loop=False, store_split=2,
           split_load0=True, last_ssp=2, pbig=False, load0_3way=True,
           xbufs=3, evbal=True):
    """bf16-wire kernel: x/y/weights in bf16, PSUM accumulate fp32.

    DMAs are batched G tiles at a time (<1 MiB transfers are descriptor-
    dominated at ~250 GB/s, 1 MiB+ reach ~340+). Host supplies x as
    [NG*P, G+1, H]: slot 0 holds the halo rows (tile i's +-HW boundary
    rows from its neighbors at partitions [16i,16i+16)), slots 1..G are
    the S-tiles, so the cross-tile coupling costs one extra K=128 matmul
    with a mostly-zero weight. Group 0's load and the last group's store
    are split so the pipeline fill/drain tails are short.
    """
    nc = bacc.Bacc("TRN2", target_bir_lowering=False, debug=False,
                   num_devices=N_CORES)
    bf = mybir.dt.bfloat16
    x_d = nc.dram_tensor("x", [NG * P, G + 1, H], bf, kind="ExternalInput")
    wc_d = nc.dram_tensor("wc", [P, P], bf, kind="ExternalInput")
    whq_d = [nc.dram_tensor(f"whq{v}", [P, P], bf, kind="ExternalInput")
             for v in range(G)]
    rn_d = nc.dram_tensor("rnorm", [P, TPB], mybir.dt.float32,
                          kind="ExternalInput")
    y_d = nc.dram_tensor("y", [NG * P, G, H], bf, kind="ExternalOutput")

    with tile.TileContext(nc) as tc:
        with (
            tc.tile_pool(name="const", bufs=1) as const_pool,
            tc.tile_pool(name="xp", bufs=xbufs) as x_pool,
            tc.tile_pool(name="op", bufs=3) as out_pool,
            tc.tile_pool(name="ps", bufs=(4 if pbig else 8),
                         space="PSUM") as psum_pool,
        ):
            wc = const_pool.tile([P, P], bf, tag="wc")
            whq = [const_pool.tile([P, P], bf, name=f"whq{v}", tag=f"whq{v}")
                   for v in range(G)]
            rn = const_pool.tile([P, TPB], mybir.dt.float32, tag="rn")
            nc.sync.dma_start(wc[:], wc_d.ap()[:])
            for v in range(G):
                nc.sync.dma_start(whq[v][:], whq_d[v].ap()[:])
            nc.sync.dma_start(rn[:], rn_d.ap()[:])

            x_groups = {}

            def load_group(g, split_first=False):
                xt = x_pool.tile([P, G + 1, H], bf, tag="x")
                if split_first and load0_3way:
                    for a, b in ((0, 2), (2, 5), (5, G + 1)):
                        nc.sync.dma_start(xt[:, a:b],
                                          x_d.ap()[g * P:(g + 1) * P, a:b])
                elif split_first:
                    # [halo + tiles 0-2] first so compute starts early
                    nc.sync.dma_start(xt[:, :4],
                                      x_d.ap()[g * P:(g + 1) * P, :4])
                    nc.sync.dma_start(xt[:, 4:],
                                      x_d.ap()[g * P:(g + 1) * P, 4:])
                else:
                    nc.sync.dma_start(xt[:], x_d.ap()[g * P:(g + 1) * P])
                x_groups[g] = xt

            def compute(t, ot):
                tt = t % TPB  # S-tile index within its batch
                i = t % G     # slot within the group
                xg = x_groups[t // G]
                whx = whq[i]
                nch = H if pbig else NCH
                for h in range(H // nch):
                    sl = slice(h * nch, (h + 1) * nch)
                    pt = psum_pool.tile([P, nch], mybir.dt.float32, tag="p")
                    nc.tensor.matmul(pt[:], wc[:], xg[:, 1 + i, sl],
                                     start=True, stop=False)
                    nc.tensor.matmul(pt[:], whx[:], xg[:, 0, sl],
                                     start=False, stop=True)
                    # evict + normalize; alternate engines to balance load
                    if pbig:
                        hf = slice(0, NCH)
                        hb = slice(NCH, H)
                        nc.vector.tensor_scalar_mul(ot[:, i, hf], pt[:, hf],
                                                    rn[:, tt:tt + 1])
                        nc.scalar.mul(ot[:, i, hb], pt[:, hb],
                                      rn[:, tt:tt + 1])
                    elif h == 0 or (evbal and t % 4 == 3):
                        nc.vector.tensor_scalar_mul(ot[:, i, sl], pt[:],
                                                    rn[:, tt:tt + 1])
                    else:
                        nc.scalar.mul(ot[:, i, sl], pt[:], rn[:, tt:tt + 1])

            def one_pass():
                x_groups.clear()
                load_group(0, split_first=split_load0)
                load_group(1)
                for g in range(NG):
                    if g + 2 < NG:
                        load_group(g + 2)
                    ot = out_pool.tile([P, G, H], bf, tag="o")
                    for i in range(G):
                        compute(g * G + i, ot)
                    # store on the scalar HWDGE ring to overlap with
                    # loads; last group in pieces for a short drain tail
                    ssp = last_ssp if g == NG - 1 else store_split
                    gs = G // ssp
                    for s in range(ssp):
                        nc.scalar.dma_start(
                            y_d.ap()[g * P:(g + 1) * P,
                                     s * gs:(s + 1) * gs],
                            ot[:, s * gs:(s + 1) * gs],
                        )

            if hw_loop and repeat > 1:
                # repeat as a hardware loop: program stays body-sized, so
                # huge R for clean slope timing without compile blowup
                mult = 1 if isinstance(hw_loop, bool) else int(hw_loop)
                with tc.For_i(0, repeat, 1):
                    for _ in range(mult):
                        one_pass()
            else:
                for _ in range(repeat):
                    one_pass()

    nc.compile()
    return nc


def _build_f32(repeat=1, hw_loop=False):
    """Plain-fp32 fallback (exact wire format, no bf16)."""
    nc = bacc.Bacc("TRN2", target_bir_lowering=False, debug=False,
                   num_devices=N_CORES)
    f32 = mybir.dt.float32
    x_d = nc.dram_tensor("x", [T * P, H], f32, kind="ExternalInput")
    wc_d = nc.dram_tensor("wc", [P, P], f32, kind="ExternalInput")
    wp_d = nc.dram_tensor("wp", [P, P], f32, kind="ExternalInput")
    wn_d = nc.dram_tensor("wn", [P, P], f32, kind="ExternalInput")
    rn_d = nc.dram_tensor("rnorm", [P, TPB], f32, kind="ExternalInput")
    y_d = nc.dram_tensor("y", [T * P, H], f32, kind="ExternalOutput")

    with tile.TileContext(nc) as tc:
        with (
            tc.tile_pool(name="const", bufs=1) as const_pool,
            tc.tile_pool(name="xp", bufs=6) as x_pool,
            tc.tile_pool(name="op", bufs=8) as out_pool,
            tc.tile_pool(name="ps", bufs=(4 if pbig else 8),
                         space="PSUM") as psum_pool,
        ):
            wc = const_pool.tile([P, P], f32, tag="wc")
            wp = const_pool.tile([P, P], f32, tag="wp")
            wn = const_pool.tile([P, P], f32, tag="wn")
            rn = const_pool.tile([P, TPB], f32, tag="rn")
            nc.sync.dma_start(wc[:], wc_d.ap()[:])
            nc.sync.dma_start(wp[:], wp_d.ap()[:])
            nc.sync.dma_start(wn[:], wn_d.ap()[:])
            nc.sync.dma_start(rn[:], rn_d.ap()[:])

            x_tiles = {}

            def load(t):
                xt = x_pool.tile([P, H], f32, tag="x")
                nc.sync.dma_start(xt[:], x_d.ap()[t * P:(t + 1) * P, :])
                x_tiles[t] = xt

            def compute(t):
                tt = t % TPB
                mms = [(wc, x_tiles[t])]
                if tt != 0:
                    mms.append((wp, x_tiles[t - 1]))
                if tt != TPB - 1:
                    mms.append((wn, x_tiles[t + 1]))
                ot = out_pool.tile([P, H], f32, tag="o")
                for h in range(HCH):
                    sl = slice(h * NCH, (h + 1) * NCH)
                    pt = psum_pool.tile([P, NCH], f32, tag="p")
                    for i, (w, xt_) in enumerate(mms):
                        nc.tensor.matmul(pt[:], w[:], xt_[:, sl],
                                         start=(i == 0),
                                         stop=(i == len(mms) - 1))
                    if h == 0:
                        nc.vector.tensor_scalar_mul(ot[:, sl], pt[:],
                                                    rn[:, tt:tt + 1])
                    else:
                        nc.scalar.mul(ot[:, sl], pt[:], rn[:, tt:tt + 1])
                nc.sync.dma_start(y_d.ap()[t * P:(t + 1) * P, :], ot[:])

            def one_pass():
                x_tiles.clear()
                load(0)
                load(1)
                for t in range(T):
                    if t + 2 < T:
                        load(t + 2)
                    compute(t)

            if hw_loop and repeat > 1:
                with tc.For_i(0, repeat, 1):
                    one_pass()
            else:
                for _ in range(repeat):
                    one_pass()

    nc.compile()
    return nc


_NC = None


def _get_nc():
    global _NC
    if _NC is None:
        _NC = _build()
    return _NC


def _in_maps(batch):
    wc, _, _, rn = _weights(BF16)
    whq = [w.astype(BF16) for w in _halo_weight_full8()]
    maps = []
    for c in range(N_CORES):
        flat = batch[c * BPC:(c + 1) * BPC].reshape(T * P, H)
        # group-major [NG, P, G, H] (16 KiB contiguous per partition per
        # group load)
        xg = flat.reshape(NG, G, P, H).transpose(0, 2, 1, 3)
        # halo slot: tile i's rows [16i,16i+8) = prev tile's last HW rows,
        # [16i+8,16i+16) = next tile's first HW rows (zero at batch edges)
        halo = np.zeros((NG, P, 1, H), dtype=flat.dtype)
        for t in range(T):
            g, i = divmod(t, G)
            if t % TPB != 0:
                halo[g, 16 * i:16 * i + HW, 0] = flat[t * P - HW:t * P]
            if t % TPB != TPB - 1:
                halo[g, 16 * i + HW:16 * i + 2 * HW, 0] = (
                    flat[(t + 1) * P:(t + 1) * P + HW]
                )
        x = np.concatenate([halo, xg], axis=2).reshape(
            NG * P, G + 1, H).astype(BF16)
        m = {"x": x, "wc": wc, "rnorm": rn}
        for v in range(G):
            m[f"whq{v}"] = whq[v]
        maps.append(m)
    return maps


def _in_maps_f32(batch):
    wc, wp, wn, rn = _weights(np.float32)
    maps = []
    for c in range(N_CORES):
        shard = np.ascontiguousarray(
            batch[c * BPC:(c + 1) * BPC].reshape(T * P, H), dtype=np.float32
        )
        maps.append({"x": shard, "wc": wc, "wp": wp, "wn": wn, "rnorm": rn})
    return maps


def kernel(batch, _trace=False):
    batch = np.asarray(batch, dtype=np.float32)
    assert batch.shape == (B, S, H), batch.shape
    res = None
    last_err = None
    # attempt 0-1: fast bf16-wire kernel; attempt 2: plain-fp32 fallback
    for attempt in range(3):
        try:
            if attempt < 2:
                nc = _get_nc()
                maps = _in_maps(batch)
            else:
                nc = _build_f32()
                maps = _in_maps_f32(batch)
            res = run_bass_kernel_spmd(nc, maps, list(range(N_CORES)),
                                       trace=_trace)
            break
        except Exception as e:  # transient device wedge: retry
            last_err = e
            global _NC
            _NC = None
    if res is None:
        raise last_err
    out = np.empty((B, S, H), dtype=np.float32)
    for c in range(N_CORES):
        y = res.results[c]["y"]
        if y.shape == (NG * P, G, H):  # grouped bf16 layout -> [T*P, H]
            y = (
                y.reshape(NG, P, G, H)
                .transpose(0, 2, 1, 3)
                .astype(np.float32)
            )
        else:  # fp32 fallback layout
            y = y.astype(np.float32)
        out[c * BPC:(c + 1) * BPC] = y.reshape(BPC, S, H)
    if _trace:
        return out, res
    return out


# revision 23
# speedup vs baseline: 2.7703x; 1.0031x over previous
"""TRN2 Bass kernel for nn_DecayModel: bidirectional decay scan (d=0.5).

Math: out[i] = (fwd[i] + bwd[i]) / norm[i] where
  fwd[i] = sum_{k<=i} d^{i-k} x[k],  bwd[i] = sum_{k>=i} d^{k-i} x[k]
  => fwd + bwd = sum_k d^{|i-k|} x[k] + x[i]
  norm[i] = (2 - d^i) + (2 - d^{S-1-i}) = 4 - d^i - d^{S-1-i}

Since d = 0.5, d^j = 2^-j decays below significance within ~30 steps, so
the scan is a banded (Toeplitz) convolution along S, computed as matmuls
over 128-row S-tiles: per tile, one full K=128 center matmul (Wc, exact
intra-tile kernel) plus one K=128 "halo" matmul whose weight selects the
16 host-gathered neighbor boundary rows (halfwidth 8; d^9+ dropped,
~3e-4). All matmuls are uniform full-array ops - interleaving sub-tile
(K=64 quadrant) matmuls with full ones measured ~2x slower. fp32 PSUM
accumulation; 1/norm per-partition scale on eviction (DVE/ACT split
~40/24 to shorten the tail).

The correctness gate is rel_err < 2e-2, so the HBM wire format is bf16
(weights are exact powers of two; fp32 accumulation): halves both DMA
traffic and PE streaming time vs fp32. Host converts fp32->bf16 on the
way in and bf16->fp32 on the way out. Measured solo per-pass: ~46 us,
vs ~43.5 us pure load+store of the same bytes (SBUF-fabric floor).

Sharding: data-parallel over batch. B=16 across 8 cores -> 2 batches/core,
flattened to [4096, 1024] (32 S-tiles; tiles 0-15 batch 0, 16-31 batch 1).
"""
import sys

sys.path.insert(0, "/opt/trn_rl_repo")

import ml_dtypes
import numpy as np

import concourse.bass as bass
import concourse.tile as tile
from concourse import bacc, mybir
from concourse.bass_utils import run_bass_kernel_spmd

DECAY = 0.5
B, S, H = 16, 2048, 1024
N_CORES = 8
BPC = B // N_CORES          # batches per core
P = 128                     # S-tile rows (partitions)
TPB = S // P                # S-tiles per batch (16)
T = BPC * TPB               # S-tiles per core (32)
NCH = 512                   # matmul moving free-dim (1 PSUM bank of fp32)
HCH = H // NCH              # H chunks per tile (2)

BF16 = ml_dtypes.bfloat16


def _weights(np_dtype):
    """Constant numpy weights: Wc/Wp/Wn lhsT matrices + 1/norm table."""
    a = np.arange(P)
    # center: M_c[a,b] = d^|a-b| + delta(a,b); symmetric so lhsT == M_c
    wc = DECAY ** np.abs(a[:, None] - a[None, :]) + np.eye(P)
    # prev tile: M_p[a,b] = d^(P+a-b); lhsT_prev[b,a] = M_p[a,b]
    wp_lhsT = DECAY ** (P + a[None, :] - a[:, None])  # [b, a]
    # next tile: M_n[a,b] = d^(P+b-a); lhsT_next[b,a] = M_n[a,b] = wp_lhsT.T
    wn_lhsT = wp_lhsT.T.copy()
    # zero negligible entries (powers of two stay exact in bf16)
    for w in (wc, wp_lhsT, wn_lhsT):
        w[w < 2.0**-60] = 0.0
    i = np.arange(S, dtype=np.float64)
    norm = 4.0 - DECAY**i - DECAY ** (S - 1.0 - i)
    rnorm = (1.0 / norm).astype(np.float32)  # [S]
    # [P, TPB]: column j = S-tile j within a batch, row a = position in tile
    rnorm_pt = rnorm.reshape(TPB, P).T.copy()
    return (
        wc.astype(np_dtype),
        wp_lhsT.astype(np_dtype),
        wn_lhsT.astype(np_dtype),
        rnorm_pt,
    )


G = 8                       # S-tiles per DMA group (1 MiB per transfer)
NG = T // G                 # groups per core (4)
HW = 8                      # halo halfwidth (cross-tile band; d^9+ dropped)


def _halo_weight():
    """[2*HW, P] lhsT: halo row j's contribution to output row a.
    j in [0,HW): prev-tile tail row at offset j-HW  -> d^(a+HW-j)
    j in [HW,2HW): next-tile head row at offset P+j-HW -> d^(P+j-HW-a)."""
    j = np.arange(2 * HW)[:, None]
    a = np.arange(P)[None, :]
    wh = np.where(
        j < HW,
        DECAY ** (a + HW - j),
        DECAY ** (P + j - HW - a),
    )
    wh[wh < 2.0**-60] = 0.0
    return wh


def _halo_weight_full8():
    """Full K=128 halo weights, one per tile slot i: rows 16i..16i+16
    carry the halo block, the rest are zero. Keeping every matmul a
    uniform full-array K=128 op avoids PE tiling-mode switches (measured
    ~2x slower when sub-tile and full matmuls interleave)."""
    wh = _halo_weight()  # [16, P]
    out = []
    for v in range(G):
        blk = np.zeros((P, P))
        blk[16 * v:16 * v + 2 * HW] = wh
        out.append(blk)
    return out


def _build(repeat=1, hw_loop=False, store_split=2,
           split_load0=True, last_ssp=2, pbig=False, load0_3way=True,
           xbufs=3, evbal=True):
    """bf16-wire kernel: x/y/weights in bf16, PSUM accumulate fp32.

    DMAs are batched G tiles at a time (<1 MiB transfers are descriptor-
    dominated at ~250 GB/s, 1 MiB+ reach ~340+). Host supplies x as
    [NG*P, G+1, H]: slot 0 holds the halo rows (tile i's +-HW boundary
    rows from its neighbors at partitions [16i,16i+16)), slots 1..G are
    the S-tiles, so the cross-tile coupling costs one extra K=128 matmul
    with a mostly-zero weight. Group 0's load and the last group's store
    are split so the pipeline fill/drain tails are short.
    """
    nc = bacc.Bacc("TRN2", target_bir_lowering=False, debug=False,
                   num_devices=N_CORES)
    bf = mybir.dt.bfloat16
    x_d = nc.dram_tensor("x", [NG * P, G + 1, H], bf, kind="ExternalInput")
    wc_d = nc.dram_tensor("wc", [P, P], bf, kind="ExternalInput")
    whq_d = [nc.dram_tensor(f"whq{v}", [P, P], bf, kind="ExternalInput")
             for v in range(G)]
    rn_d = nc.dram_tensor("rnorm", [P, TPB], mybir.dt.float32,
                          kind="ExternalInput")
    y_d = nc.dram_tensor("y", [NG * P, G, H], bf, kind="ExternalOutput")

    with tile.TileContext(nc) as tc:
        with (
            tc.tile_pool(name="const", bufs=1) as const_pool,
            tc.tile_pool(name="xp", bufs=xbufs) as x_pool,
            tc.tile_pool(name="op", bufs=3) as out_pool,
            tc.tile_pool(name="ps", bufs=(4 if pbig else 8),
                         space="PSUM") as psum_pool,
        ):
            wc = const_pool.tile([P, P], bf, tag="wc")
            whq = [const_pool.tile([P, P], bf, name=f"whq{v}", tag=f"whq{v}")
                   for v in range(G)]
            rn = const_pool.tile([P, TPB], mybir.dt.float32, tag="rn")
            nc.sync.dma_start(wc[:], wc_d.ap()[:])
            for v in range(G):
                nc.sync.dma_start(whq[v][:], whq_d[v].ap()[:])
            nc.sync.dma_start(rn[:], rn_d.ap()[:])

            x_groups = {}

            def load_group(g, split_first=False):
                xt = x_pool.tile([P, G + 1, H], bf, tag="x")
                if split_first and load0_3way:
                    for a, b in ((0, 2), (2, 5), (5, G + 1)):
                        nc.sync.dma_start(xt[:, a:b],
                                          x_d.ap()[g * P:(g + 1) * P, a:b])
                elif split_first:
                    # [halo + tiles 0-2] first so compute starts early
                    nc.sync.dma_start(xt[:, :4],
                                      x_d.ap()[g * P:(g + 1) * P, :4])
                    nc.sync.dma_start(xt[:, 4:],
                                      x_d.ap()[g * P:(g + 1) * P, 4:])
                else:
                    nc.sync.dma_start(xt[:], x_d.ap()[g * P:(g + 1) * P])
                x_groups[g] = xt

            def compute(t, ot):
                tt = t % TPB  # S-tile index within its batch
                i = t % G     # slot within the group
                xg = x_groups[t // G]
                whx = whq[i]
                nch = H if pbig else NCH
                for h in range(H // nch):
                    sl = slice(h * nch, (h + 1) * nch)
                    pt = psum_pool.tile([P, nch], mybir.dt.float32, tag="p")
                    nc.tensor.matmul(pt[:], wc[:], xg[:, 1 + i, sl],
                                     start=True, stop=False)
                    nc.tensor.matmul(pt[:], whx[:], xg[:, 0, sl],
                                     start=False, stop=True)
                    # evict + normalize; alternate engines to balance load
                    if pbig:
                        hf = slice(0, NCH)
                        hb = slice(NCH, H)
                        nc.vector.tensor_scalar_mul(ot[:, i, hf], pt[:, hf],
                                                    rn[:, tt:tt + 1])
                        nc.scalar.mul(ot[:, i, hb], pt[:, hb],
                                      rn[:, tt:tt + 1])
                    elif h == 0 or (evbal and t % 4 == 3):
                        nc.vector.tensor_scalar_mul(ot[:, i, sl], pt[:],
                                                    rn[:, tt:tt + 1])
                    else:
                        nc.scalar.mul(ot[:, i, sl], pt[:], rn[:, tt:tt + 1])

            def one_pass():
                x_groups.clear()
                load_group(0, split_first=split_load0)
                load_group(1)
                for g in range(NG):
                    if g + 2 < NG:
                        load_group(g + 2)
                    ot = out_pool.tile([P, G, H], bf, tag="o")
                    for i in range(G):
                        compute(g * G + i, ot)
                    # store on the scalar HWDGE ring to overlap with
                    # loads; last group in pieces for a short drain tail
                    ssp = last_ssp if g == NG - 1 else store_split
                    gs = G // ssp
                    for s in range(ssp):
                        nc.scalar.dma_start(
                            y_d.ap()[g * P:(g + 1) * P,
                                     s * gs:(s + 1) * gs],
                            ot[:, s * gs:(s + 1) * gs],
                        )

            if hw_loop and repeat > 1:
                # repeat as a hardware loop: program stays body-sized, so
                # huge R for clean slope timing without compile blowup
                mult = 1 if isinstance(hw_loop, bool) else int(hw_loop)
                with tc.For_i(0, repeat, 1):
                    for _ in range(mult):
                        one_pass()
            else:
                for _ in range(repeat):
                    one_pass()

    nc.compile()
    return nc


def _build_f32(repeat=1, hw_loop=False):
    """Plain-fp32 fallback (exact wire format, no bf16)."""
    nc = bacc.Bacc("TRN2", target_bir_lowering=False, debug=False,
                   num_devices=N_CORES)
    f32 = mybir.dt.float32
    x_d = nc.dram_tensor("x", [T * P, H], f32, kind="ExternalInput")
    wc_d = nc.dram_tensor("wc", [P, P], f32, kind="ExternalInput")
    wp_d = nc.dram_tensor("wp", [P, P], f32, kind="ExternalInput")
    wn_d = nc.dram_tensor("wn", [P, P], f32, kind="ExternalInput")
    rn_d = nc.dram_tensor("rnorm", [P, TPB], f32, kind="ExternalInput")
    y_d = nc.dram_tensor("y", [T * P, H], f32, kind="ExternalOutput")

    with tile.TileContext(nc) as tc:
        with (
            tc.tile_pool(name="const", bufs=1) as const_pool,
            tc.tile_pool(name="xp", bufs=6) as x_pool,
            tc.tile_pool(name="op", bufs=8) as out_pool,
            tc.tile_pool(name="ps", bufs=8, space="PSUM") as psum_pool,
        ):
            wc = const_pool.tile([P, P], f32, tag="wc")
            wp = const_pool.tile([P, P], f32, tag="wp")
            wn = const_pool.tile([P, P], f32, tag="wn")
            rn = const_pool.tile([P, TPB], f32, tag="rn")
            nc.sync.dma_start(wc[:], wc_d.ap()[:])
            nc.sync.dma_start(wp[:], wp_d.ap()[:])
            nc.sync.dma_start(wn[:], wn_d.ap()[:])
            nc.sync.dma_start(rn[:], rn_d.ap()[:])

            x_tiles = {}

            def load(t):
                xt = x_pool.tile([P, H], f32, tag="x")
                nc.sync.dma_start(xt[:], x_d.ap()[t * P:(t + 1) * P, :])
                x_tiles[t] = xt

            def compute(t):
                tt = t % TPB
                mms = [(wc, x_tiles[t])]
                if tt != 0:
                    mms.append((wp, x_tiles[t - 1]))
                if tt != TPB - 1:
                    mms.append((wn, x_tiles[t + 1]))
                ot = out_pool.tile([P, H], f32, tag="o")
                for h in range(HCH):
                    sl = slice(h * NCH, (h + 1) * NCH)
                    pt = psum_pool.tile([P, NCH], f32, tag="p")
                    for i, (w, xt_) in enumerate(mms):
                        nc.tensor.matmul(pt[:], w[:], xt_[:, sl],
                                         start=(i == 0),
                                         stop=(i == len(mms) - 1))
                    if h == 0:
                        nc.vector.tensor_scalar_mul(ot[:, sl], pt[:],
                                                    rn[:, tt:tt + 1])
                    else:
                        nc.scalar.mul(ot[:, sl], pt[:], rn[:, tt:tt + 1])
                nc.sync.dma_start(y_d.ap()[t * P:(t + 1) * P, :], ot[:])

            def one_pass():
                x_tiles.clear()
                load(0)
                load(1)
                for t in range(T):
                    if t + 2 < T:
                        load(t + 2)
                    compute(t)

            if hw_loop and repeat > 1:
                with tc.For_i(0, repeat, 1):
                    one_pass()
            else:
                for _ in range(repeat):
                    one_pass()

    nc.compile()
    return nc


_NC = None


def _get_nc():
    global _NC
    if _NC is None:
        _NC = _build()
    return _NC


def _in_maps(batch):
    wc, _, _, rn = _weights(BF16)
    whq = [w.astype(BF16) for w in _halo_weight_full8()]
    maps = []
    for c in range(N_CORES):
        flat = batch[c * BPC:(c + 1) * BPC].reshape(T * P, H)
        # group-major [NG, P, G, H] (16 KiB contiguous per partition per
        # group load)
        xg = flat.reshape(NG, G, P, H).transpose(0, 2, 1, 3)
        # halo slot: tile i's rows [16i,16i+8) = prev tile's last HW rows,
        # [16i+8,16i+16) = next tile's first HW rows (zero at batch edges)
        halo = np.zeros((NG, P, 1, H), dtype=flat.dtype)
        for t in range(T):
            g, i = divmod(t, G)
            if t % TPB != 0:
                halo[g, 16 * i:16 * i + HW, 0] = flat[t * P - HW:t * P]
            if t % TPB != TPB - 1:
                halo[g, 16 * i + HW:16 * i + 2 * HW, 0] = (
                    flat[(t + 1) * P:(t + 1) * P + HW]
                )
        x = np.concatenate([halo, xg], axis=2).reshape(
            NG * P, G + 1, H).astype(BF16)
        m = {"x": x, "wc": wc, "rnorm": rn}
        for v in range(G):
            m[f"whq{v}"] = whq[v]
        maps.append(m)
    return maps


def _in_maps_f32(batch):
    wc, wp, wn, rn = _weights(np.float32)
    maps = []
    for c in range(N_CORES):
        shard = np.ascontiguousarray(
            batch[c * BPC:(c + 1) * BPC].reshape(T * P, H), dtype=np.float32
        )
        maps.append({"x": shard, "wc": wc, "wp": wp, "wn": wn, "rnorm": rn})
    return maps


def kernel(batch, _trace=False):
    batch = np.asarray(batch, dtype=np.float32)
    assert batch.shape == (B, S, H), batch.shape
    res = None
    last_err = None
    # attempt 0-1: fast bf16-wire kernel; attempt 2: plain-fp32 fallback
    for attempt in range(3):
        try:
            if attempt < 2:
                nc = _get_nc()
                maps = _in_maps(batch)
            else:
                nc = _build_f32()
                maps = _in_maps_f32(batch)
            res = run_bass_kernel_spmd(nc, maps, list(range(N_CORES)),
                                       trace=_trace)
            break
        except Exception as e:  # transient device wedge: retry
            last_err = e
            global _NC
            _NC = None
    if res is None:
        raise last_err
    out = np.empty((B, S, H), dtype=np.float32)
    for c in range(N_CORES):
        y = res.results[c]["y"]
        if y.shape == (NG * P, G, H):  # grouped bf16 layout -> [T*P, H]
            y = (
                y.reshape(NG, P, G, H)
                .transpose(0, 2, 1, 3)
                .astype(np.float32)
            )
        else:  # fp32 fallback layout
            y = y.astype(np.float32)
        out[c * BPC:(c + 1) * BPC] = y.reshape(BPC, S, H)
    if _trace:
        return out, res
    return out
